# revision 16
# baseline (speedup 1.0000x reference)
"""Trainium2 Bass kernel for NT-Xent contrastive loss (BATCH=4096, DIM=512, TEMP=0.5).

fp8 DoubleRow, per-core column-rotated, replicated norms, engine-balanced:
  - Host: E = concat(emb_i, emb_j) [8192, 512] f32. fp8e4 transpose image
    (rotated so each core's own block is group 0) feeds the matmuls; a
    bf16 row-major image (same rotation) feeds norms/positives.
  - Device (per core, SPMD, no cross-core sync):
      * sumsq of all rows, split DVE scalar_tensor_tensor / ACT Square
        (Square, Exp, Ln share one activation-table set)
      * r = 1/||e|| = exp(-0.5*ln(ss)); alpha*r broadcast down partitions
        with PE (transpose + selector matmul)
      * normalize column image to fp8 zq = Eq*r*alpha (DVE k=0, GpSimd k>0)
      * S' = zq_own^T @ zq on PE in fp8 perf_mode=DoubleRow; ACT computes
        exp(S'/(alpha^2*T)) with fused row-sum accumulation; groups
        (0)(1)(23)(45)(67) per row-tile; prep ops for group c+2 are
        interleaved into group c's emission to keep every queue fed
      * positives via DVE row-dots of own (group 0) x partner (group 4)
      * per-core partial: sum_rows(log(den - e^{1/T}) - pos/T) -> [1,1]
  - Host: loss = sum(partials) / (2B).
"""

import math

import ml_dtypes
import numpy as np

BATCH = 4096
DIM = 512
TEMP = 0.5
B2 = 2 * BATCH              # 8192 rows/cols of the similarity matrix
NCORES = 8
RPC = B2 // NCORES          # 1024 rows per core
KT = DIM // 128             # 4 contraction chunks of 128
CG = 8                      # column groups
CGW = B2 // CG              # 1024 columns per group
T8 = RPC // 128             # 8 row-tiles per core
NG = 5                      # main groups per row-tile: (0)(1)(23)(45)(67)
ALPHA = 16.0                # fp8 scale for normalized operands
ASCALE = 1.0 / (ALPHA * ALPHA * TEMP)
EXP_DIAG = math.exp(1.0 / TEMP)

_CACHE = {}


def _build():
    import concourse.bacc as bacc
    import concourse.mybir as mybir
    import concourse.tile as tile

    f32 = mybir.dt.float32
    bf16 = mybir.dt.bfloat16
    fp8 = mybir.dt.float8e4
    AF = mybir.ActivationFunctionType
    ALU = mybir.AluOpType
    X = mybir.AxisListType.X
    DR = mybir.MatmulPerfMode.DoubleRow

    import bass_rust as _bass_rust
    from concourse.hw_specs import get_activation_tables

    class _Bacc(bacc.Bacc):
        """Pin Exp+Ln+Square to the natural_log_exp_and_others table set."""

        def insert_act_table_loads(self):
            has_activation = any(
                isinstance(i, mybir.InstActivation)
                for b in self.main_func.blocks
                for i in b.instructions)
            if not has_activation:
                return
            drop = {mybir.ActivationFunctionType.Exp,
                    mybir.ActivationFunctionType.Ln,
                    mybir.ActivationFunctionType.Square}
            tables = []
            for name, funcs in get_activation_tables(self.m.arch).items():
                if name != "natural_log_exp_and_others":
                    funcs = funcs - drop
                tables.append((name, funcs))
            _bass_rust.insert_act_table_loads(self, tables)

    nc = _Bacc("TRN2", target_bir_lowering=False, debug=False,
               num_devices=NCORES)

    et_d = nc.dram_tensor("et", [DIM, B2], fp8, kind="ExternalInput").ap()
    erm_d = nc.dram_tensor("erm", [128, (B2 // 128) * DIM], bf16,
                           kind="ExternalInput").ap()
    iden_d = nc.dram_tensor("iden", [128, 128], bf16, kind="ExternalInput").ap()
    sel_d = nc.dram_tensor("sel", [128, T8 * 128], bf16,
                           kind="ExternalInput").ap()
    out_d = nc.dram_tensor("out", [1, 1], f32, kind="ExternalOutput").ap()

    with tile.TileContext(nc) as tc:
        with (
            tc.tile_pool(name="persist", bufs=1) as P,
            tc.tile_pool(name="scratch", bufs=2) as S,
            tc.tile_pool(name="psum", bufs=2, space="PSUM") as PS,
        ):
            ssg = [P.tile([128, T8], f32, name=f"ss_{c}") for c in range(CG)]
            ssA0 = P.tile([128, 4], f32, name="ssA0")
            ssV0 = P.tile([128, 4], f32, name="ssV0")
            rawpos = P.tile([128, T8], f32, name="rawpos")
            rsums = P.tile([128, T8 * NG], f32, name="rsums")
            rb8 = P.tile([128, T8], f32, name="rb8")
            rp8 = P.tile([128, T8], f32, name="rp8")
            pos8 = P.tile([128, T8], f32, name="pos8")
            ones = P.tile([128, 1], f32, name="ones")
            iden = P.tile([128, 128], bf16, name="iden")
            sel = P.tile([128, T8 * 128], bf16, name="sel")
            erm = [P.tile([128, T8 * DIM], bf16, name=f"erm_{c}")
                   for c in range(CG)]
            etn = [P.tile([128, KT, CGW], fp8, name=f"etn_{c}")
                   for c in range(CG)]
            et3 = [None] * CG

            nc.vector.memset(ones[:], 1.0)
            nc.sync.dma_start(iden[:], iden_d[:])
            nc.sync.dma_start(sel[:], sel_d[:])

            def load_c(c):
                nc.sync.dma_start(erm[c][:], erm_d[:, c * T8 * DIM:
                                                   (c + 1) * T8 * DIM])
                et3[c] = S.tile([128, KT, CGW], fp8, name=f"et_{c}",
                                tag="etraw", bufs=4)
                for k in range(KT):
                    nc.sync.dma_start(
                        et3[c][:, k, :],
                        et_d[k * 128:(k + 1) * 128, c * CGW:(c + 1) * CGW])

            def sq_act(c, t, dst):
                sqo = S.tile([128, DIM], bf16, tag="sqo", name="sqo")
                nc.scalar.activation(
                    sqo[:], erm[c][:, t * DIM:(t + 1) * DIM], AF.Square,
                    accum_out=dst)

            def sq_dve(c, t, dst):
                sco = S.tile([128, DIM], bf16, tag="sttv", name="sco")
                src = erm[c][:, t * DIM:(t + 1) * DIM]
                nc.vector.scalar_tensor_tensor(
                    sco[:], src, 1.0, src, ALU.mult, ALU.mult,
                    accum_out=dst)

            def sumsq_ops(c):
                """Closures computing sumsq of group c on a private ss
                tile, one engine per group (ACT for c==1, DVE otherwise;
                c==0 is split across both via two sub-tiles)."""
                ops = []
                for t in range(T8):
                    if c == 0:
                        if t < 4:
                            ops.append(lambda t=t: sq_act(
                                0, t, ssA0[:, t:t + 1]))
                        else:
                            ops.append(lambda t=t: sq_dve(
                                0, t, ssV0[:, t - 4:t - 3]))
                    elif c == 1:
                        ops.append(lambda t=t: sq_act(
                            1, t, ssg[1][:, t:t + 1]))
                    else:
                        ops.append(lambda c=c, t=t: sq_dve(
                            c, t, ssg[c][:, t:t + 1]))
                return ops

            def rsqrt(dst_ap, src_ap, w):
                """dst = 1/sqrt(src) via exp(-0.5*ln(x)) on ACT."""
                ln = S.tile([128, w], f32, tag=f"ln{w}", name="ln")
                nc.scalar.activation(ln[:], src_ap, AF.Ln)
                nc.scalar.activation(dst_ap, ln[:], AF.Exp, scale=-0.5)

            def rchain(c):
                """alpha*r for group c broadcast down partitions -> rbc,
                then normalize et3[c] -> etn[c] (fp8)."""
                rcb = S.tile([128, 128], bf16, tag="rcb", name="rcb")
                nc.vector.memset(rcb[:], 0.0)
                if c == 0:
                    rsqrt(rcb[:, 0:4], ssA0[:], 4)
                    rsqrt(rcb[:, 4:T8], ssV0[:], 4)
                    nc.vector.tensor_copy(rb8[:], rcb[:, 0:T8])
                else:
                    rsqrt(rcb[:, 0:T8], ssg[c][:], T8)
                ptr = PS.tile([128, 128], bf16, tag="mm", name="ptr")
                nc.tensor.transpose(ptr[:], rcb[:], iden[:])
                rT = S.tile([128, 128], bf16, tag="rT", name="rT")
                nc.vector.tensor_copy(rT[:], ptr[:])
                pb = PS.tile([128, CGW], f32, tag="mm", name="pb")
                for t in range(T8):
                    nc.tensor.matmul(pb[:, t * 128:(t + 1) * 128],
                                     sel[:, t * 128:(t + 1) * 128],
                                     rT[:], start=True, stop=True)
                rbc = S.tile([128, CGW], f32, tag="rbc", name="rbc", bufs=2)
                nc.vector.tensor_copy(rbc[:], pb[:])
                for k in range(KT):
                    eng = nc.vector if k == 0 else nc.gpsimd
                    eng.tensor_tensor(etn[c][:, k, :], et3[c][:, k, :],
                                      rbc[:], ALU.mult)

            def main_group(gi, cgs, prep):
                """fp8 DoubleRow matmul + exp/accum for column groups cgs
                over all 8 row-tiles, draining prep closures between
                tiles. Stationary = own block (etn[0] slices)."""
                wid = len(cgs) * CGW
                per_tile = -(-len(prep) // T8) if prep else 0
                for t in range(T8):
                    ps = PS.tile([128, wid], f32, tag="mm", name="psmm")
                    for k2 in range(KT // 2):
                        ksl = slice(2 * k2, 2 * k2 + 2)
                        for ci, c in enumerate(cgs):
                            for n in range(CGW // 512):
                                lo = ci * CGW + n * 512
                                nc.tensor.matmul(
                                    ps[:, lo:lo + 512],
                                    etn[0][:, ksl, t * 128:(t + 1) * 128],
                                    etn[c][:, ksl, n * 512:(n + 1) * 512],
                                    start=(k2 == 0), stop=(k2 == KT // 2 - 1),
                                    perf_mode=DR)
                    sce = S.tile([128, wid], bf16, tag="expout", name="sce")
                    col = t * NG + gi
                    nc.scalar.activation(sce[:], ps[:], AF.Exp, scale=ASCALE,
                                         accum_out=rsums[:, col:col + 1])
                    for _ in range(per_tile):
                        if prep:
                            prep.pop(0)()

            def positives_ops():
                ops = []
                for t in range(T8):
                    def op(t=t):
                        sco = S.tile([128, DIM], bf16, tag="sttv", name="scop")
                        nc.vector.scalar_tensor_tensor(
                            sco[:], erm[0][:, t * DIM:(t + 1) * DIM], 1.0,
                            erm[4][:, t * DIM:(t + 1) * DIM],
                            ALU.mult, ALU.mult,
                            accum_out=rawpos[:, t:t + 1])
                    ops.append(op)

                def fin():
                    rsqrt(rp8[:], ssg[4][:], T8)
                    pt0 = P.tile([128, T8], f32, name="pt0")
                    nc.vector.tensor_mul(pt0[:], rawpos[:], rb8[:])
                    pt1 = P.tile([128, T8], f32, name="pt1")
                    nc.vector.tensor_mul(pt1[:], pt0[:], rp8[:])
                    nc.vector.tensor_scalar_mul(pos8[:], pt1[:], 1.0 / TEMP)
                ops.append(fin)
                return ops

            # ---- paced emission ----
            load_c(0)
            load_c(1)
            load_c(2)
            for op in sumsq_ops(0):
                op()
            rchain(0)
            for op in sumsq_ops(1):
                op()
            rchain(1)
            load_c(3)
            load_c(4)

            prep = sumsq_ops(2) + sumsq_ops(3)
            main_group(0, (0,), prep)
            rchain(2)
            rchain(3)
            load_c(5)
            load_c(6)

            prep = sumsq_ops(4) + sumsq_ops(5)
            main_group(1, (1,), prep)
            rchain(4)
            rchain(5)
            load_c(7)

            prep = sumsq_ops(6) + positives_ops()
            main_group(2, (2, 3), prep)
            rchain(6)

            prep = sumsq_ops(7)
            main_group(3, (4, 5), prep)
            rchain(7)

            main_group(4, (6, 7), [])

            # ---- finalize: den = rowsum - e^{1/T}; sum(log(den) - pos) ----
            den8 = P.tile([128, T8], f32, name="den8")
            nc.vector.tensor_reduce(
                den8[:], rsums[:].rearrange("p (t c) -> p t c", c=NG),
                X, ALU.add)
            den8b = P.tile([128, T8], f32, name="den8b")
            nc.vector.tensor_scalar_add(den8b[:], den8[:], -EXP_DIAG)
            logd = S.tile([128, T8], f32, tag="logd", name="logd")
            tlog = P.tile([128, 1], f32, name="tlog")
            nc.scalar.activation(logd[:], den8b[:], AF.Ln, accum_out=tlog[:])
            tpos = P.tile([128, 1], f32, name="tpos")
            nc.vector.tensor_reduce(tpos[:], pos8[:], X, ALU.add)
            lv = P.tile([128, 1], f32, name="lv")
            nc.vector.tensor_sub(lv[:], tlog[:], tpos[:])
            psf = PS.tile([1, 1], f32, tag="mm", name="psf")
            nc.tensor.matmul(psf[:], lv[:], ones[:], start=True, stop=True)
            ob = P.tile([1, 1], f32, name="ob")
            nc.vector.tensor_copy(ob[:], psf[:])
            nc.sync.dma_start(out_d[:], ob[:])

    nc.compile()
    return nc


def _get_nc():
    if "nc" not in _CACHE:
        _CACHE["nc"] = _build()
    return _CACHE["nc"]


def _in_maps(emb_i, emb_j):
    bf = ml_dtypes.bfloat16
    f8 = ml_dtypes.float8_e4m3
    E = np.concatenate([np.asarray(emb_i, dtype=np.float32),
                        np.asarray(emb_j, dtype=np.float32)], axis=0)
    Eq = E.astype(f8)
    ETu8 = np.ascontiguousarray(Eq.view(np.uint8).T)    # [512, 8192]
    Ebf = E.astype(bf)                                  # [8192, 512]
    Ebfu16 = Ebf.view(np.uint16)
    SEL = np.zeros((128, T8 * 128), dtype=bf)
    for tp in range(T8):
        SEL[tp, tp * 128:(tp + 1) * 128] = ALPHA
    IDEN = np.eye(128, dtype=bf)
    maps = []
    for k in range(NCORES):
        s = k * RPC
        et_rot = np.ascontiguousarray(np.roll(ETu8, -s, axis=1))
        Er = np.roll(Ebfu16, -s, axis=0)
        ermr = np.ascontiguousarray(
            Er.reshape(B2 // 128, 128, DIM).transpose(1, 0, 2).reshape(128, -1))
        maps.append({
            "et": et_rot.view(f8),
            "erm": ermr.view(bf),
            "iden": IDEN,
            "sel": SEL,
        })
    return maps


def _run(emb_i, emb_j, trace=False):
    from concourse.bass_utils import run_bass_kernel_spmd
    nc = _get_nc()
    res = run_bass_kernel_spmd(nc, _in_maps(emb_i, emb_j),
                               list(range(NCORES)), trace=trace)
    total = sum(float(res.results[i]["out"][0, 0]) for i in range(NCORES))
    loss = np.float32(total / B2)
    return loss, res


def kernel(emb_i, emb_j):
    return _run(emb_i, emb_j, trace=False)[0]


# revision 19
# speedup vs baseline: 1.2004x; 1.2004x over previous
"""Trainium2 Bass kernel for NT-Xent contrastive loss (BATCH=4096, DIM=512, TEMP=0.5).

fp8 DoubleRow, per-core column-rotated, replicated norms, engine-balanced:
  - Host: E = concat(emb_i, emb_j) [8192, 512] f32. fp8e4 transpose image
    (rotated so each core's own block is group 0) feeds the matmuls; a
    bf16 row-major image (same rotation) feeds norms/positives.
  - Device (per core, SPMD, no cross-core sync):
      * sumsq of all rows, split DVE scalar_tensor_tensor / ACT Square
        (Square, Exp, Ln share one activation-table set)
      * r = 1/||e|| = exp(-0.5*ln(ss)); alpha*r broadcast down partitions
        with PE (transpose + selector matmul)
      * normalize column image to fp8 zq = Eq*r*alpha (DVE k=0, GpSimd k>0)
      * S' = zq_own^T @ zq on PE in fp8 perf_mode=DoubleRow; ACT computes
        exp(S'/(alpha^2*T)) with fused row-sum accumulation; groups
        (0)(1)(23)(45)(67) per row-tile; prep ops for group c+2 are
        interleaved into group c's emission to keep every queue fed
      * positives via DVE row-dots of own (group 0) x partner (group 4)
      * per-core partial: sum_rows(log(den - e^{1/T}) - pos/T) -> [1,1]
  - Host: loss = sum(partials) / (2B).
"""

import math

import ml_dtypes
import numpy as np

BATCH = 4096
DIM = 512
TEMP = 0.5
B2 = 2 * BATCH              # 8192 rows/cols of the similarity matrix
NCORES = 8
RPC = B2 // NCORES          # 1024 rows per core
KT = DIM // 128             # 4 contraction chunks of 128
CG = 8                      # column groups
CGW = B2 // CG              # 1024 columns per group
T8 = RPC // 128             # 8 row-tiles per core
NG = 5                      # main groups per row-tile: (0)(1)(23)(45)(67)
ALPHA = 16.0                # fp8 scale for normalized operands
ASCALE = 1.0 / (ALPHA * ALPHA * TEMP)
EXP_DIAG = math.exp(1.0 / TEMP)

_CACHE = {}


def _build():
    import concourse.bacc as bacc
    import concourse.mybir as mybir
    import concourse.tile as tile

    f32 = mybir.dt.float32
    bf16 = mybir.dt.bfloat16
    fp8 = mybir.dt.float8e4
    AF = mybir.ActivationFunctionType
    ALU = mybir.AluOpType
    X = mybir.AxisListType.X
    DR = mybir.MatmulPerfMode.DoubleRow

    import bass_rust as _bass_rust
    from concourse.hw_specs import get_activation_tables

    class _Bacc(bacc.Bacc):
        """Pin Exp+Ln+Square to the natural_log_exp_and_others table set."""

        def insert_act_table_loads(self):
            has_activation = any(
                isinstance(i, mybir.InstActivation)
                for b in self.main_func.blocks
                for i in b.instructions)
            if not has_activation:
                return
            drop = {mybir.ActivationFunctionType.Exp,
                    mybir.ActivationFunctionType.Ln,
                    mybir.ActivationFunctionType.Square}
            tables = []
            for name, funcs in get_activation_tables(self.m.arch).items():
                if name != "natural_log_exp_and_others":
                    funcs = funcs - drop
                tables.append((name, funcs))
            _bass_rust.insert_act_table_loads(self, tables)

    nc = _Bacc("TRN2", target_bir_lowering=False, debug=False,
               num_devices=NCORES)

    et_d = nc.dram_tensor("et", [DIM, B2], fp8, kind="ExternalInput").ap()
    erm_d = nc.dram_tensor("erm", [128, (B2 // 128) * DIM], bf16,
                           kind="ExternalInput").ap()
    iden_d = nc.dram_tensor("iden", [128, 128], bf16, kind="ExternalInput").ap()
    sel_d = nc.dram_tensor("sel", [128, T8 * 128], bf16,
                           kind="ExternalInput").ap()
    out_d = nc.dram_tensor("out", [1, 1], f32, kind="ExternalOutput").ap()

    with tile.TileContext(nc) as tc:
        with (
            tc.tile_pool(name="persist", bufs=1) as P,
            tc.tile_pool(name="scratch", bufs=2) as S,
            tc.tile_pool(name="psum", bufs=2, space="PSUM") as PS,
        ):
            ssg = [P.tile([128, T8], f32, name=f"ss_{c}") for c in range(CG)]
            ssA0 = P.tile([128, 4], f32, name="ssA0")
            ssV0 = P.tile([128, 4], f32, name="ssV0")
            rawpos = P.tile([128, T8], f32, name="rawpos")
            rsums = P.tile([128, T8 * NG], f32, name="rsums")
            rb8 = P.tile([128, T8], f32, name="rb8")
            rp8 = P.tile([128, T8], f32, name="rp8")
            pos8 = P.tile([128, T8], f32, name="pos8")
            ones = P.tile([128, 1], f32, name="ones")
            iden = P.tile([128, 128], bf16, name="iden")
            sel = P.tile([128, T8 * 128], bf16, name="sel")
            erm = [P.tile([128, T8 * DIM], bf16, name=f"erm_{c}")
                   for c in range(CG)]
            etn = [P.tile([128, KT, CGW], fp8, name=f"etn_{c}")
                   for c in range(CG)]
            et3 = [None] * CG

            nc.vector.memset(ones[:], 1.0)
            nc.sync.dma_start(iden[:], iden_d[:])
            nc.sync.dma_start(sel[:], sel_d[:])

            def load_c(c):
                nc.sync.dma_start(erm[c][:], erm_d[:, c * T8 * DIM:
                                                   (c + 1) * T8 * DIM])
                et3[c] = S.tile([128, KT, CGW], fp8, name=f"et_{c}",
                                tag="etraw", bufs=4)
                for k in range(KT):
                    nc.sync.dma_start(
                        et3[c][:, k, :],
                        et_d[k * 128:(k + 1) * 128, c * CGW:(c + 1) * CGW])

            def sq_act(c, t, dst):
                sqo = S.tile([128, DIM], bf16, tag="sqo", name="sqo")
                nc.scalar.activation(
                    sqo[:], erm[c][:, t * DIM:(t + 1) * DIM], AF.Square,
                    accum_out=dst)

            def sq_dve(c, t, dst):
                sco = S.tile([128, DIM], bf16, tag="sttv", name="sco")
                src = erm[c][:, t * DIM:(t + 1) * DIM]
                nc.vector.scalar_tensor_tensor(
                    sco[:], src, 1.0, src, ALU.mult, ALU.mult,
                    accum_out=dst)

            def sumsq_ops(c, n_act):
                """Closures computing sumsq of group c: n_act ops on ACT
                (into ssA-subtiles for groups 0/1 to decouple engines),
                the rest on DVE."""
                ops = []
                for t in range(T8):
                    if t < n_act:
                        dst = (ssA0[:, t:t + 1] if c == 0 else
                               ssg[c][:, t:t + 1])
                        ops.append(lambda c=c, t=t, dst=dst: sq_act(
                            c, t, dst))
                    else:
                        dst = (ssV0[:, t - 4:t - 3] if c == 0 else
                               ssg[c][:, t:t + 1])
                        ops.append(lambda c=c, t=t, dst=dst: sq_dve(
                            c, t, dst))
                return ops

            def rsqrt(dst_ap, src_ap, w):
                """dst = 1/sqrt(src) via exp(-0.5*ln(x)) on ACT."""
                ln = S.tile([128, w], f32, tag=f"ln{w}", name="ln")
                nc.scalar.activation(ln[:], src_ap, AF.Ln)
                nc.scalar.activation(dst_ap, ln[:], AF.Exp, scale=-0.5)

            def rchain(c):
                """alpha*r for group c broadcast down partitions -> rbc,
                then normalize et3[c] -> etn[c] (fp8)."""
                rcb = S.tile([128, 128], bf16, tag="rcb", name="rcb")
                nc.vector.memset(rcb[:], 0.0)
                if c == 0:
                    rsqrt(rcb[:, 0:4], ssA0[:], 4)
                    rsqrt(rcb[:, 4:T8], ssV0[:], 4)
                    nc.vector.tensor_copy(rb8[:], rcb[:, 0:T8])
                else:
                    rsqrt(rcb[:, 0:T8], ssg[c][:], T8)
                ptr = PS.tile([128, 128], bf16, tag="mm", name="ptr")
                nc.tensor.transpose(ptr[:], rcb[:], iden[:])
                rT = S.tile([128, 128], bf16, tag="rT", name="rT")
                nc.vector.tensor_copy(rT[:], ptr[:])
                pb = PS.tile([128, CGW], f32, tag="mm", name="pb")
                for t in range(T8):
                    nc.tensor.matmul(pb[:, t * 128:(t + 1) * 128],
                                     sel[:, t * 128:(t + 1) * 128],
                                     rT[:], start=True, stop=True)
                rbc = S.tile([128, CGW], f32, tag="rbc", name="rbc", bufs=3)
                nc.vector.tensor_copy(rbc[:], pb[:])
                for k in range(KT):
                    eng = nc.vector if k == 0 else nc.gpsimd
                    eng.tensor_tensor(etn[c][:, k, :], et3[c][:, k, :],
                                      rbc[:], ALU.mult)

            def main_group(gi, cgs, prep):
                """fp8 DoubleRow matmul + exp/accum for column groups cgs
                over all 8 row-tiles, draining prep closures between
                tiles. Stationary = own block (etn[0] slices)."""
                wid = len(cgs) * CGW
                per_tile = -(-len(prep) // T8) if prep else 0
                for t in range(T8):
                    ps = PS.tile([128, wid], f32, tag="mm", name="psmm")
                    for k2 in range(KT // 2):
                        ksl = slice(2 * k2, 2 * k2 + 2)
                        for ci, c in enumerate(cgs):
                            for n in range(CGW // 512):
                                lo = ci * CGW + n * 512
                                nc.tensor.matmul(
                                    ps[:, lo:lo + 512],
                                    etn[0][:, ksl, t * 128:(t + 1) * 128],
                                    etn[c][:, ksl, n * 512:(n + 1) * 512],
                                    start=(k2 == 0), stop=(k2 == KT // 2 - 1),
                                    perf_mode=DR)
                    sce = S.tile([128, wid], bf16, tag="expout", name="sce")
                    col = t * NG + gi
                    nc.scalar.activation(sce[:], ps[:], AF.Exp, scale=ASCALE,
                                         accum_out=rsums[:, col:col + 1])
                    for _ in range(per_tile):
                        if prep:
                            prep.pop(0)()

            def positives_ops():
                ops = []
                for t in range(T8):
                    def op(t=t):
                        sco = S.tile([128, DIM], bf16, tag="sttv", name="scop")
                        nc.vector.scalar_tensor_tensor(
                            sco[:], erm[0][:, t * DIM:(t + 1) * DIM], 1.0,
                            erm[4][:, t * DIM:(t + 1) * DIM],
                            ALU.mult, ALU.mult,
                            accum_out=rawpos[:, t:t + 1])
                    ops.append(op)

                def fin():
                    rsqrt(rp8[:], ssg[4][:], T8)
                    pt0 = P.tile([128, T8], f32, name="pt0")
                    nc.vector.tensor_mul(pt0[:], rawpos[:], rb8[:])
                    pt1 = P.tile([128, T8], f32, name="pt1")
                    nc.vector.tensor_mul(pt1[:], pt0[:], rp8[:])
                    nc.vector.tensor_scalar_mul(pos8[:], pt1[:], 1.0 / TEMP)
                ops.append(fin)
                return ops

            # ---- paced emission ----
            load_c(0)
            load_c(1)
            load_c(2)
            for op in sumsq_ops(0, 4):
                op()
            rchain(0)
            for op in sumsq_ops(1, 4):
                op()
            rchain(1)
            load_c(3)
            load_c(4)

            prep = sumsq_ops(2, 1) + sumsq_ops(3, 1)
            main_group(0, (0,), prep)
            rchain(2)
            rchain(3)
            load_c(5)
            load_c(6)

            prep = sumsq_ops(4, 0) + sumsq_ops(5, 0)
            main_group(1, (1,), prep)
            rchain(4)
            rchain(5)
            load_c(7)

            prep = sumsq_ops(6, 0) + positives_ops()
            main_group(2, (2, 3), prep)
            rchain(6)

            prep = sumsq_ops(7, 0)
            main_group(3, (4, 5), prep)
            rchain(7)

            main_group(4, (6, 7), [])

            # ---- finalize: den = rowsum - e^{1/T}; sum(log(den) - pos) ----
            den8 = P.tile([128, T8], f32, name="den8")
            nc.vector.tensor_reduce(
                den8[:], rsums[:].rearrange("p (t c) -> p t c", c=NG),
                X, ALU.add)
            den8b = P.tile([128, T8], f32, name="den8b")
            nc.vector.tensor_scalar_add(den8b[:], den8[:], -EXP_DIAG)
            logd = S.tile([128, T8], f32, tag="logd", name="logd")
            tlog = P.tile([128, 1], f32, name="tlog")
            nc.scalar.activation(logd[:], den8b[:], AF.Ln, accum_out=tlog[:])
            tpos = P.tile([128, 1], f32, name="tpos")
            nc.vector.tensor_reduce(tpos[:], pos8[:], X, ALU.add)
            lv = P.tile([128, 1], f32, name="lv")
            nc.vector.tensor_sub(lv[:], tlog[:], tpos[:])
            psf = PS.tile([1, 1], f32, tag="mm", name="psf")
            nc.tensor.matmul(psf[:], lv[:], ones[:], start=True, stop=True)
            ob = P.tile([1, 1], f32, name="ob")
            nc.vector.tensor_copy(ob[:], psf[:])
            nc.sync.dma_start(out_d[:], ob[:])

    nc.compile()
    return nc


def _get_nc():
    if "nc" not in _CACHE:
        _CACHE["nc"] = _build()
    return _CACHE["nc"]


def _in_maps(emb_i, emb_j):
    bf = ml_dtypes.bfloat16
    f8 = ml_dtypes.float8_e4m3
    E = np.concatenate([np.asarray(emb_i, dtype=np.float32),
                        np.asarray(emb_j, dtype=np.float32)], axis=0)
    Eq = E.astype(f8)
    ETu8 = np.ascontiguousarray(Eq.view(np.uint8).T)    # [512, 8192]
    Ebf = E.astype(bf)                                  # [8192, 512]
    Ebfu16 = Ebf.view(np.uint16)
    SEL = np.zeros((128, T8 * 128), dtype=bf)
    for tp in range(T8):
        SEL[tp, tp * 128:(tp + 1) * 128] = ALPHA
    IDEN = np.eye(128, dtype=bf)
    maps = []
    for k in range(NCORES):
        s = k * RPC
        et_rot = np.ascontiguousarray(np.roll(ETu8, -s, axis=1))
        Er = np.roll(Ebfu16, -s, axis=0)
        ermr = np.ascontiguousarray(
            Er.reshape(B2 // 128, 128, DIM).transpose(1, 0, 2).reshape(128, -1))
        maps.append({
            "et": et_rot.view(f8),
            "erm": ermr.view(bf),
            "iden": IDEN,
            "sel": SEL,
        })
    return maps


def _run(emb_i, emb_j, trace=False):
    from concourse.bass_utils import run_bass_kernel_spmd
    nc = _get_nc()
    res = run_bass_kernel_spmd(nc, _in_maps(emb_i, emb_j),
                               list(range(NCORES)), trace=trace)
    total = sum(float(res.results[i]["out"][0, 0]) for i in range(NCORES))
    loss = np.float32(total / B2)
    return loss, res


def kernel(emb_i, emb_j):
    return _run(emb_i, emb_j, trace=False)[0]
